# revision 21
# baseline (speedup 1.0000x reference)
"""Trainium2 Bass kernel for CartesianLoss (v3, merged-op chunked pipeline).

Loss = mean_n min_perm mean_i ||polar2cart(target_i) - polar2cart(pred_perm(i))||_2

Pure data parallelism over the batch (N=131072) across 8 cores; each core
handles 16384 samples as (128 partitions, 128 samples). Host packs inputs
chunk-major, source-major, fp16, so every device op is contiguous fp16
(DVE 2x packed mode) with no on-device transposes.

Per-op fixed costs (~150ns DVE issue + sem waits) dominate small ops, so v3
merges aggressively:
- one trig pair per chunk (cos/sin into halves of one tile)
- coords for x/y and t/p in ONE tensor_tensor (broadcast AP over 4 planes)
- both outer-difference planes in ONE op
- both squares in ONE op, both pair outer-sum matrices (G01,G23) in ONE op
- triangle fold as 4 dual-F in-place min ops
- per-sample min tree instead of 1x tensor_reduce

Assignment (min over 120 perms) uses meet-in-the-middle: F01/F23 pair mins
via dense 5x5 outer-sum + triangle fold, g3 triples via 3 arms, combine with
reversed-rank access.
"""

import contextlib

import numpy as np

import concourse.bass as bass
import concourse.bass_isa as bass_isa
import concourse.bacc as bacc
import concourse.tile as tile
from concourse import mybir

N = 131072
M = 5
NCORES = 8
NPC = N // NCORES          # samples per core
P = 128                    # partitions
FS = NPC // P              # samples per partition (128)
HALF_PI = 1.5707963267948966

F32 = mybir.dt.float32
F16 = mybir.dt.float16
TT = mybir.AluOpType
AFT = mybir.ActivationFunctionType

# --- tunables -------------------------------------------------------------
NCH = 2                    # front-end sample chunks (divides FS)
ASN_NCH = 1                # assignment blocks (divides FS)
SQ_ENGINE = "split"        # 'act' | 'dve' | 'split' (squares of dx/dy)
ADD_ENGINE = "dve"         # 'dve' | 'gp'    (d2 = dx2 + dy2)
ARMS_ENGINE = "dve"        # 'dve' | 'gp'
G3_ENGINE = "dve"          # 'dve' | 'gp'    (3-way arm min)

TRACE = False


def _ap(t, offset_elems, dims):
    """Manual free-dim AP on tile t: dims = [[step,count],...] (elements)."""
    full = t[:]
    return bass.AP(
        tensor=full.tensor,
        offset=full.offset + offset_elems,
        ap=[full.ap[0]] + [list(d) for d in dims],
    )


def build_bass(loop_iters=None, nch=None, asn_nch=None, sq_engine=None,
               add_engine=None, arms_engine=None, g3_engine=None):
    nch = NCH if nch is None else nch
    asn_nch = ASN_NCH if asn_nch is None else asn_nch
    sq_engine = SQ_ENGINE if sq_engine is None else sq_engine
    add_engine = ADD_ENGINE if add_engine is None else add_engine
    arms_engine = ARMS_ENGINE if arms_engine is None else arms_engine
    g3_engine = G3_ENGINE if g3_engine is None else g3_engine
    CS = FS // nch
    W = FS // asn_nch
    assert FS % nch == 0 and FS % asn_nch == 0

    nc = bacc.Bacc(
        "TRN2", target_bir_lowering=False, debug=False, num_devices=NCORES
    )
    hpi_t = nc.alloc_sbuf_tensor("const-float32-hpi", [P, 1], F32)
    nc.gpsimd.memset(hpi_t.ap(), HALF_PI)
    nc.const_aps.aps[(F32, HALF_PI)] = hpi_t.ap()
    nc.all_engine_barrier()

    in_d = nc.dram_tensor("inp", [P, nch, 2, 2, M, CS], F16, kind="ExternalInput")
    out_d = nc.dram_tensor("partials", [1, 1], F32, kind="ExternalOutput")

    gp = nc.gpsimd
    dve = nc.vector
    eng = {"dve": dve, "gp": gp}
    MCS = M * CS

    with tile.TileContext(nc) as tc:
        with contextlib.ExitStack() as stack:
            if loop_iters is not None:
                stack.enter_context(tc.For_i(0, loop_iters, 1))
            pool = stack.enter_context(tc.tile_pool(name="main", bufs=1))

            def tl(shape, dt, tag):
                return pool.tile(shape, dt, name="t", tag=tag)

            IN = [tl([P, 2, 2, M, CS], F16, f"in{c}") for c in range(nch)]
            ang = [t[:, 0] for t in IN]
            dst = [t[:, 1] for t in IN]
            # TRIG[h]: h=0 cos, h=1 sin, each [2(t/p), M, CS]
            TRIG = [tl([P, 2, 2, M, CS], F16, f"trig{c}") for c in range(nch)]
            CRD = [tl([P, 2, 2, M, CS], F16, f"crd{c}") for c in range(nch)]
            DXY = [tl([P, 2, M, M, CS], F16, f"dxy{c}") for c in range(nch)]
            SQ = [tl([P, 2, M, M, CS], F16, f"sq{c}") for c in range(nch)]
            D2 = [tl([P, M * M, CS], F16, f"d2{c}") for c in range(nch)]
            D = tl([P, M * M, FS], F16, "dfull")
            GT = [tl([P, 2, M, M, W], F16, f"gt{b}") for b in range(asn_nch)]
            At = [tl([P, 10, W], F16, f"at{b}") for b in range(asn_nch)]
            Ar = [tl([P, 10, W], F16, f"ar{b}") for b in range(asn_nch)]
            Aq = [tl([P, 10, W], F16, f"aq{b}") for b in range(asn_nch)]
            G3 = [tl([P, 10, W], F16, f"g3_{b}") for b in range(asn_nch)]
            ANS = [tl([P, 10, W], F16, f"ans{b}") for b in range(asn_nch)]
            T1 = [tl([P, M, W], F16, f"t1_{b}") for b in range(asn_nch)]
            T2 = [tl([P, 2, W], F16, f"t2_{b}") for b in range(asn_nch)]
            T3 = [tl([P, 1, W], F16, f"t3_{b}") for b in range(asn_nch)]
            RES = tl([P, asn_nch, W], F32, "res")
            PART = tl([P, 1], F32, "part")
            PARTR = tl([P, 1], F32, "partr")

            # ---- DMA: one transfer per chunk (ang+dst packed) ----
            for c in range(nch):
                nc.sync.dma_start(out=IN[c][:], in_=in_d[:, c])

            # ---- ACT trig: all chunks first (one table set) ----
            for c in range(nch):
                nc.scalar.activation(TRIG[c][:, 0], ang[c], AFT.Sin, bias=HALF_PI)
                nc.scalar.activation(TRIG[c][:, 1], ang[c], AFT.Sin)


            # ---- DVE front-end per chunk ----
            for c in range(nch):
                # CRD[h,tp,m,s] = TRIG[h,tp,m,s] * dst[tp,m,s], split per h
                # (h=0 needs only cos, so DVE starts right after the first
                # ACT op), each followed by its DXY outer-difference plane
                for h in range(2):
                    dve.tensor_tensor(
                        _ap(CRD[c], h * 2 * MCS, [[MCS, 2], [1, MCS]]),
                        _ap(TRIG[c], h * 2 * MCS, [[MCS, 2], [1, MCS]]),
                        _ap(IN[c], 2 * MCS, [[MCS, 2], [1, MCS]]),
                        TT.mult,
                    )
                    dve.tensor_tensor(
                        DXY[c][:, h],
                        _ap(CRD[c], h * 2 * MCS, [[CS, M], [0, M], [1, CS]]),
                        _ap(CRD[c], h * 2 * MCS + MCS, [[0, M], [CS, M], [1, CS]]),
                        TT.subtract,
                    )
                if sq_engine == "dve":
                    dve.tensor_tensor(SQ[c][:], DXY[c][:], DXY[c][:], TT.mult)

            # ---- squares: y-half on ACT (emitted first so ACT streams on),
            #      x-half on DVE when 'split'; d2 add on DVE; sqrt split into
            #      rows 0-3 (gates pair stage) and row 4 (gates arms only) ----
            for c in range(nch):
                if sq_engine == "act":
                    nc.scalar.activation(SQ[c][:], DXY[c][:], AFT.Square)
                elif sq_engine == "split":
                    nc.scalar.activation(SQ[c][:, 1], DXY[c][:, 1], AFT.Square)
            if sq_engine == "split":
                for c in range(nch):
                    dve.tensor_tensor(
                        SQ[c][:, 0], DXY[c][:, 0], DXY[c][:, 0], TT.mult
                    )
            for c in range(nch):
                eng[add_engine].tensor_tensor(
                    D2[c][:], SQ[c][:, 0], SQ[c][:, 1], TT.add
                )
            for r0, nrows in ((0, 10), (10, 10), (20, 5)):
                for c in range(nch):
                    nc.scalar.activation(
                        _ap(D, r0 * FS + c * CS, [[FS, nrows], [1, CS]]),
                        D2[c][:, r0:r0 + nrows], AFT.Sqrt,
                    )

            # ---- assignment per block b over D[:, :, b*W:(b+1)*W] ----
            if arms_engine == "split":
                # arm_t on GpSimd, concurrent with DVE's arm_r/arm_q; g3
                # consumes At last (min(min(Ar,Aq), At)) so the slower Gp
                # path hides behind the DVE ops
                arm_engines = (gp, dve, dve)
            else:
                arm_engines = (eng[arms_engine],) * 3
            g3e = eng[g3_engine]

            for b in range(asn_nch):
                off = b * W
                Gt = GT[b]
                # outer-sums: Gt[f,a,b,s] = D[r0(f),a,s] + D[r1(f),b,s]
                # rows (0,1) for f=0 -> F01, rows (2,3) for f=1 -> F23
                for f, (r0, r1) in enumerate(((0, 1), (2, 3))):
                    dve.tensor_tensor(
                        Gt[:, f],
                        _ap(D, off + r0 * 5 * FS, [[FS, M], [0, M], [1, W]]),
                        _ap(D, off + r1 * 5 * FS, [[0, M], [FS, M], [1, W]]),
                        TT.add,
                    )
                # dual triangle fold: F[a<b] = min(G[a,b], G[b,a]), in place
                for a in range(4):
                    n = 4 - a
                    dve.tensor_tensor(
                        _ap(Gt, (6 * a + 1) * W, [[25 * W, 2], [W, n], [1, W]]),
                        _ap(Gt, (6 * a + 1) * W, [[25 * W, 2], [W, n], [1, W]]),
                        _ap(Gt, ((a + 1) * M + a) * W, [[25 * W, 2], [M * W, n], [1, W]]),
                        TT.min,
                    )

                # F23 = Gt[:,1] (offset 25W, slot stride W); D4 = D row 4
                F23o = 25 * W

                def f23(s0, dims):
                    return _ap(Gt, F23o + s0 * W, dims)

                def d4(j0, dims):
                    return _ap(D, off + (20 + j0) * FS, dims)

                e_t, e_r, e_q = arm_engines
                # arm_t: j = t. At[T] = F23[q,r] + D4[t]
                e_t.tensor_tensor(At[b][:, 0:3, :], f23(1, [[0, 3], [1, W]]),
                                  d4(2, [[FS, 3], [1, W]]), TT.add)
                e_t.tensor_tensor(
                    _ap(At[b], 3 * W, [[3 * W, 2], [W, 2], [1, W]]),
                    f23(2, [[5 * W, 2], [0, 2], [1, W]]),
                    d4(3, [[0, 2], [FS, 2], [1, W]]), TT.add)
                e_t.tensor_tensor(
                    _ap(At[b], 5 * W, [[3 * W, 2], [1, W]]),
                    f23(3, [[5 * W, 2], [1, W]]),
                    d4(4, [[0, 2], [1, W]]), TT.add)
                e_t.tensor_tensor(At[b][:, 9:10, :], f23(13, [[0, 1], [1, W]]),
                                  d4(4, [[0, 1], [1, W]]), TT.add)
                # arm_r: j = r. Ar[T] = F23[q,t] + D4[r]
                e_r.tensor_tensor(Ar[b][:, 0:3, :], f23(2, [[W, 3], [1, W]]),
                                  d4(1, [[0, 3], [1, W]]), TT.add)
                e_r.tensor_tensor(
                    _ap(Ar[b], 3 * W, [[3 * W, 2], [W, 2], [1, W]]),
                    f23(3, [[5 * W, 2], [W, 2], [1, W]]),
                    d4(2, [[0, 2], [0, 2], [1, W]]), TT.add)
                e_r.tensor_tensor(
                    _ap(Ar[b], 5 * W, [[3 * W, 2], [1, W]]),
                    f23(4, [[5 * W, 2], [1, W]]),
                    d4(3, [[0, 2], [1, W]]), TT.add)
                e_r.tensor_tensor(Ar[b][:, 9:10, :], f23(14, [[0, 1], [1, W]]),
                                  d4(3, [[0, 1], [1, W]]), TT.add)
                # arm_q: j = q. Aq[T] = F23[r,t] + D4[q]
                e_q.tensor_tensor(Aq[b][:, 0:3, :], f23(7, [[W, 3], [1, W]]),
                                  d4(0, [[0, 3], [1, W]]), TT.add)
                e_q.tensor_tensor(
                    _ap(Aq[b], 3 * W, [[3 * W, 2], [W, 2], [1, W]]),
                    f23(13, [[0, 2], [W, 2], [1, W]]),
                    d4(0, [[FS, 2], [0, 2], [1, W]]), TT.add)
                e_q.tensor_tensor(
                    _ap(Aq[b], 5 * W, [[3 * W, 2], [1, W]]),
                    f23(19, [[0, 2], [1, W]]),
                    d4(0, [[FS, 2], [1, W]]), TT.add)
                e_q.tensor_tensor(Aq[b][:, 9:10, :], f23(19, [[0, 1], [1, W]]),
                                  d4(2, [[0, 1], [1, W]]), TT.add)

                g3e.tensor_tensor(G3[b][:], Ar[b][:], Aq[b][:], TT.min)
                g3e.tensor_tensor(G3[b][:], G3[b][:], At[b][:], TT.min)

                # combine: ans[k] = F01[pair k] + g3[9-k]; F01 = Gt[:,0]
                dve.tensor_tensor(
                    ANS[b][:, 0:4, :], _ap(Gt, 1 * W, [[W, 4], [1, W]]),
                    _ap(G3[b], 9 * W, [[-W, 4], [1, W]]), TT.add)
                dve.tensor_tensor(
                    ANS[b][:, 4:7, :], _ap(Gt, 7 * W, [[W, 3], [1, W]]),
                    _ap(G3[b], 5 * W, [[-W, 3], [1, W]]), TT.add)
                dve.tensor_tensor(
                    ANS[b][:, 7:9, :], _ap(Gt, 13 * W, [[W, 2], [1, W]]),
                    _ap(G3[b], 2 * W, [[-W, 2], [1, W]]), TT.add)
                dve.tensor_tensor(
                    ANS[b][:, 9:10, :], _ap(Gt, 19 * W, [[0, 1], [1, W]]),
                    G3[b][:, 0:1, :], TT.add)
                # min tree over the 10 slots
                dve.tensor_tensor(T1[b][:], ANS[b][:, 0:5, :], ANS[b][:, 5:10, :], TT.min)
                dve.tensor_tensor(T2[b][:], T1[b][:, 0:2, :], T1[b][:, 2:4, :], TT.min)
                dve.tensor_tensor(T3[b][:], T2[b][:, 0:1, :], T2[b][:, 1:2, :], TT.min)
                dve.tensor_tensor(RES[:, b], T3[b][:, 0, :], T1[b][:, 4, :], TT.min)

            dve.tensor_reduce(
                PART[:], _ap(RES, 0, [[1, asn_nch * W]]),
                mybir.AxisListType.X, TT.add,
            )
            # collapse to one partition so the output DMA is a single-queue
            # 4-byte transfer (a [128,1] source fans out over 16 queues whose
            # completion crawl costs ~7us at the tail)
            gp.partition_all_reduce(
                PARTR[:], PART[:], 128, bass_isa.ReduceOp.add
            )
            nc.sync.dma_start(out=out_d[:], in_=PARTR[0:1, :])

    nc.compile()
    return nc


_CACHED_RUNNER = None


def _pack_inputs(ta, pa, td, pd, nch):
    """(N, M) f32 x4 -> (NCORES*P, nch, 2, 2, M, CS) f16, chunk-major."""
    CS = FS // nch
    out = np.empty((NCORES * P, nch, 2, 2, M, CS), np.float16)
    for k, (a, b) in enumerate(((ta, pa), (td, pd))):
        a5 = np.asarray(a, np.float32).reshape(NCORES * P, nch, CS, M)
        b5 = np.asarray(b, np.float32).reshape(NCORES * P, nch, CS, M)
        out[:, :, k, 0] = a5.transpose(0, 1, 3, 2)
        out[:, :, k, 1] = b5.transpose(0, 1, 3, 2)
    return out


def _make_runner():
    import jax
    from jax.sharding import Mesh, NamedSharding, PartitionSpec
    from jax.experimental.shard_map import shard_map
    from concourse.bass2jax import (
        _bass_exec_p, install_neuronx_cc_hook, partition_id_tensor,
    )

    nc = build_bass()
    install_neuronx_cc_hook()
    partition_name = nc.partition_id_tensor.name if nc.partition_id_tensor else None
    in_names, out_names, out_avals, zero_outs = [], [], [], []
    for alloc in nc.m.functions[0].allocations:
        if not isinstance(alloc, mybir.MemoryLocationSet):
            continue
        name = alloc.memorylocations[0].name
        if alloc.kind == "ExternalInput":
            if name != partition_name:
                in_names.append(name)
        elif alloc.kind == "ExternalOutput":
            shape = tuple(alloc.tensor_shape)
            dtype = mybir.dt.np(alloc.dtype)
            out_names.append(name)
            out_avals.append(jax.core.ShapedArray(shape, dtype))
            zero_outs.append(np.zeros(shape, dtype))
    n_params = len(in_names)
    all_in_names = in_names + out_names
    if partition_name is not None:
        all_in_names = all_in_names + [partition_name]

    def _body(*args):
        operands = list(args)
        if partition_name is not None:
            operands.append(partition_id_tensor())
        return tuple(_bass_exec_p.bind(
            *operands,
            out_avals=tuple(out_avals),
            in_names=tuple(all_in_names),
            out_names=tuple(out_names),
            lowering_input_output_aliases=(),
            sim_require_finite=True,
            sim_require_nnan=True,
            nc=nc,
        ))

    devices = jax.devices()[:NCORES]
    mesh = Mesh(np.asarray(devices), ("core",))
    in_specs = (PartitionSpec("core"),) * (n_params + len(out_names))
    out_specs = (PartitionSpec("core"),) * len(out_names)
    fn = jax.jit(
        shard_map(_body, mesh=mesh, in_specs=in_specs, out_specs=out_specs,
                  check_rep=False),
        keep_unused=True,
    )
    sharding = NamedSharding(mesh, PartitionSpec("core"))
    concat_zeros = [
        np.zeros((NCORES * z.shape[0], *z.shape[1:]), z.dtype) for z in zero_outs
    ]
    zeros_dev = [jax.device_put(z, sharding) for z in concat_zeros]

    def run(inputs_by_name):
        import jax as _jax
        args = [
            _jax.device_put(np.ascontiguousarray(inputs_by_name[nm]), sharding)
            for nm in in_names
        ]
        outs = fn(*args, *zeros_dev)
        return {nm: np.asarray(outs[i]) for i, nm in enumerate(out_names)}

    return run


def kernel(predictions_angle, targets_angle, predictions_distance, targets_distance):
    global _CACHED_RUNNER
    if _CACHED_RUNNER is None:
        _CACHED_RUNNER = _make_runner()
    out = _CACHED_RUNNER({
        "inp": _pack_inputs(targets_angle, predictions_angle,
                            targets_distance, predictions_distance, NCH),
    })
    total = out["partials"].astype(np.float64).sum()
    return np.asarray(total / N / M, dtype=np.float32)


# revision 22
# speedup vs baseline: 1.0403x; 1.0403x over previous
"""Trainium2 Bass kernel for CartesianLoss (v3, merged-op chunked pipeline).

Loss = mean_n min_perm mean_i ||polar2cart(target_i) - polar2cart(pred_perm(i))||_2

Pure data parallelism over the batch (N=131072) across 8 cores; each core
handles 16384 samples as (128 partitions, 128 samples). Host packs inputs
chunk-major, source-major, fp16, so every device op is contiguous fp16
(DVE 2x packed mode) with no on-device transposes.

Per-op fixed costs (~150ns DVE issue + sem waits) dominate small ops, so v3
merges aggressively:
- one trig pair per chunk (cos/sin into halves of one tile)
- coords for x/y and t/p in ONE tensor_tensor (broadcast AP over 4 planes)
- both outer-difference planes in ONE op
- both squares in ONE op, both pair outer-sum matrices (G01,G23) in ONE op
- triangle fold as 4 dual-F in-place min ops
- per-sample min tree instead of 1x tensor_reduce

Assignment (min over 120 perms) uses meet-in-the-middle: F01/F23 pair mins
via dense 5x5 outer-sum + triangle fold, g3 triples via 3 arms, combine with
reversed-rank access.
"""

import contextlib

import numpy as np

import concourse.bass as bass
import concourse.bass_isa as bass_isa
import concourse.bacc as bacc
import concourse.tile as tile
from concourse import mybir

N = 131072
M = 5
NCORES = 8
NPC = N // NCORES          # samples per core
P = 128                    # partitions
FS = NPC // P              # samples per partition (128)
HALF_PI = 1.5707963267948966

F32 = mybir.dt.float32
F16 = mybir.dt.float16
TT = mybir.AluOpType
AFT = mybir.ActivationFunctionType

# --- tunables -------------------------------------------------------------
NCH = 2                    # front-end sample chunks (divides FS)
ASN_NCH = 1                # assignment blocks (divides FS)
SQ_ENGINE = "split"        # 'act' | 'dve' | 'split' (squares of dx/dy)
ADD_ENGINE = "dve"         # 'dve' | 'gp'    (d2 = dx2 + dy2)
ARMS_ENGINE = "dve"        # 'dve' | 'gp'
G3_ENGINE = "dve"          # 'dve' | 'gp'    (3-way arm min)

TRACE = False


def _ap(t, offset_elems, dims):
    """Manual free-dim AP on tile t: dims = [[step,count],...] (elements)."""
    full = t[:]
    return bass.AP(
        tensor=full.tensor,
        offset=full.offset + offset_elems,
        ap=[full.ap[0]] + [list(d) for d in dims],
    )


def build_bass(loop_iters=None, nch=None, asn_nch=None, sq_engine=None,
               add_engine=None, arms_engine=None, g3_engine=None):
    nch = NCH if nch is None else nch
    asn_nch = ASN_NCH if asn_nch is None else asn_nch
    sq_engine = SQ_ENGINE if sq_engine is None else sq_engine
    add_engine = ADD_ENGINE if add_engine is None else add_engine
    arms_engine = ARMS_ENGINE if arms_engine is None else arms_engine
    g3_engine = G3_ENGINE if g3_engine is None else g3_engine
    CS = FS // nch
    W = FS // asn_nch
    assert FS % nch == 0 and FS % asn_nch == 0

    nc = bacc.Bacc(
        "TRN2", target_bir_lowering=False, debug=False, num_devices=NCORES
    )
    hpi_t = nc.alloc_sbuf_tensor("const-float32-hpi", [P, 1], F32)
    nc.gpsimd.memset(hpi_t.ap(), HALF_PI)
    nc.const_aps.aps[(F32, HALF_PI)] = hpi_t.ap()
    nc.all_engine_barrier()

    in_d = nc.dram_tensor("inp", [P, nch, 2, 2, M, CS], F16, kind="ExternalInput")
    out_d = nc.dram_tensor("partials", [1, 1], F32, kind="ExternalOutput")

    gp = nc.gpsimd
    dve = nc.vector
    eng = {"dve": dve, "gp": gp}
    MCS = M * CS

    with tile.TileContext(nc) as tc:
        with contextlib.ExitStack() as stack:
            if loop_iters is not None:
                stack.enter_context(tc.For_i(0, loop_iters, 1))
            pool = stack.enter_context(tc.tile_pool(name="main", bufs=1))

            def tl(shape, dt, tag):
                return pool.tile(shape, dt, name="t", tag=tag)

            IN = [tl([P, 2, 2, M, CS], F16, f"in{c}") for c in range(nch)]
            ang = [t[:, 0] for t in IN]
            dst = [t[:, 1] for t in IN]
            # TRIG[h]: h=0 cos, h=1 sin, each [2(t/p), M, CS]
            TRIG = [tl([P, 2, 2, M, CS], F16, f"trig{c}") for c in range(nch)]
            CRD = [tl([P, 2, 2, M, CS], F16, f"crd{c}") for c in range(nch)]
            DXY = [tl([P, 2, M, M, CS], F16, f"dxy{c}") for c in range(nch)]
            SQ = [tl([P, 2, M, M, CS], F16, f"sq{c}") for c in range(nch)]
            D2 = [tl([P, M * M, CS], F16, f"d2{c}") for c in range(nch)]
            D = tl([P, M * M, FS], F16, "dfull")
            GT = [tl([P, 2, M, M, W], F16, f"gt{b}") for b in range(asn_nch)]
            At = [tl([P, 10, W], F16, f"at{b}") for b in range(asn_nch)]
            Ar = [tl([P, 10, W], F16, f"ar{b}") for b in range(asn_nch)]
            Aq = [tl([P, 10, W], F16, f"aq{b}") for b in range(asn_nch)]
            G3 = [tl([P, 10, W], F16, f"g3_{b}") for b in range(asn_nch)]
            ANS = [tl([P, 10, W], F16, f"ans{b}") for b in range(asn_nch)]
            T1 = [tl([P, M, W], F16, f"t1_{b}") for b in range(asn_nch)]
            T2 = [tl([P, 2, W], F16, f"t2_{b}") for b in range(asn_nch)]
            T3 = [tl([P, 1, W], F16, f"t3_{b}") for b in range(asn_nch)]
            RES = tl([P, asn_nch, W], F32, "res")
            PART = tl([P, 1], F32, "part")
            PARTR = tl([P, 1], F32, "partr")

            # ---- DMA: one transfer per chunk (ang+dst packed) ----
            for c in range(nch):
                nc.sync.dma_start(out=IN[c][:], in_=in_d[:, c])

            # ---- ACT trig: all chunks first (one table set) ----
            for c in range(nch):
                nc.scalar.activation(TRIG[c][:, 0], ang[c], AFT.Sin, bias=HALF_PI)
                nc.scalar.activation(TRIG[c][:, 1], ang[c], AFT.Sin)


            # ---- DVE front-end per chunk ----
            for c in range(nch):
                # CRD[h,tp,m,s] = TRIG[h,tp,m,s] * dst[tp,m,s]. Chunk 0 is
                # split per h so DVE starts right after the first ACT op
                # (cos0); later chunks' trig is long done, so one merged op.
                if c == 0:
                    for h in range(2):
                        dve.tensor_tensor(
                            _ap(CRD[c], h * 2 * MCS, [[MCS, 2], [1, MCS]]),
                            _ap(TRIG[c], h * 2 * MCS, [[MCS, 2], [1, MCS]]),
                            _ap(IN[c], 2 * MCS, [[MCS, 2], [1, MCS]]),
                            TT.mult,
                        )
                        dve.tensor_tensor(
                            DXY[c][:, h],
                            _ap(CRD[c], h * 2 * MCS, [[CS, M], [0, M], [1, CS]]),
                            _ap(CRD[c], h * 2 * MCS + MCS,
                                [[0, M], [CS, M], [1, CS]]),
                            TT.subtract,
                        )
                else:
                    dve.tensor_tensor(
                        _ap(CRD[c], 0, [[2 * MCS, 2], [MCS, 2], [1, MCS]]),
                        _ap(TRIG[c], 0, [[2 * MCS, 2], [MCS, 2], [1, MCS]]),
                        _ap(IN[c], 2 * MCS, [[0, 2], [MCS, 2], [1, MCS]]),
                        TT.mult,
                    )
                    for h in range(2):
                        dve.tensor_tensor(
                            DXY[c][:, h],
                            _ap(CRD[c], h * 2 * MCS, [[CS, M], [0, M], [1, CS]]),
                            _ap(CRD[c], h * 2 * MCS + MCS,
                                [[0, M], [CS, M], [1, CS]]),
                            TT.subtract,
                        )
                if sq_engine == "dve":
                    dve.tensor_tensor(SQ[c][:], DXY[c][:], DXY[c][:], TT.mult)

            # ---- squares: y-half on ACT (emitted first so ACT streams on),
            #      x-half on DVE when 'split'; d2 add on DVE; sqrt split into
            #      rows 0-3 (gates pair stage) and row 4 (gates arms only) ----
            for c in range(nch):
                if sq_engine == "act":
                    nc.scalar.activation(SQ[c][:], DXY[c][:], AFT.Square)
                elif sq_engine == "split":
                    nc.scalar.activation(SQ[c][:, 1], DXY[c][:, 1], AFT.Square)
            if sq_engine == "split":
                for c in range(nch):
                    dve.tensor_tensor(
                        SQ[c][:, 0], DXY[c][:, 0], DXY[c][:, 0], TT.mult
                    )
            for c in range(nch):
                eng[add_engine].tensor_tensor(
                    D2[c][:], SQ[c][:, 0], SQ[c][:, 1], TT.add
                )
            for r0, nrows in ((0, 10), (10, 10), (20, 5)):
                for c in range(nch):
                    nc.scalar.activation(
                        _ap(D, r0 * FS + c * CS, [[FS, nrows], [1, CS]]),
                        D2[c][:, r0:r0 + nrows], AFT.Sqrt,
                    )

            # ---- assignment per block b over D[:, :, b*W:(b+1)*W] ----
            if arms_engine == "split":
                # arm_t on GpSimd, concurrent with DVE's arm_r/arm_q; g3
                # consumes At last (min(min(Ar,Aq), At)) so the slower Gp
                # path hides behind the DVE ops
                arm_engines = (gp, dve, dve)
            else:
                arm_engines = (eng[arms_engine],) * 3
            g3e = eng[g3_engine]

            for b in range(asn_nch):
                off = b * W
                Gt = GT[b]
                # outer-sums: Gt[f,a,b,s] = D[r0(f),a,s] + D[r1(f),b,s]
                # rows (0,1) for f=0 -> F01, rows (2,3) for f=1 -> F23
                for f, (r0, r1) in enumerate(((0, 1), (2, 3))):
                    dve.tensor_tensor(
                        Gt[:, f],
                        _ap(D, off + r0 * 5 * FS, [[FS, M], [0, M], [1, W]]),
                        _ap(D, off + r1 * 5 * FS, [[0, M], [FS, M], [1, W]]),
                        TT.add,
                    )
                # dual triangle fold: F[a<b] = min(G[a,b], G[b,a]), in place
                for a in range(4):
                    n = 4 - a
                    dve.tensor_tensor(
                        _ap(Gt, (6 * a + 1) * W, [[25 * W, 2], [W, n], [1, W]]),
                        _ap(Gt, (6 * a + 1) * W, [[25 * W, 2], [W, n], [1, W]]),
                        _ap(Gt, ((a + 1) * M + a) * W, [[25 * W, 2], [M * W, n], [1, W]]),
                        TT.min,
                    )

                # F23 = Gt[:,1] (offset 25W, slot stride W); D4 = D row 4
                F23o = 25 * W

                def f23(s0, dims):
                    return _ap(Gt, F23o + s0 * W, dims)

                def d4(j0, dims):
                    return _ap(D, off + (20 + j0) * FS, dims)

                e_t, e_r, e_q = arm_engines
                # arm_t: j = t. At[T] = F23[q,r] + D4[t]
                e_t.tensor_tensor(At[b][:, 0:3, :], f23(1, [[0, 3], [1, W]]),
                                  d4(2, [[FS, 3], [1, W]]), TT.add)
                e_t.tensor_tensor(
                    _ap(At[b], 3 * W, [[3 * W, 2], [W, 2], [1, W]]),
                    f23(2, [[5 * W, 2], [0, 2], [1, W]]),
                    d4(3, [[0, 2], [FS, 2], [1, W]]), TT.add)
                e_t.tensor_tensor(
                    _ap(At[b], 5 * W, [[3 * W, 2], [1, W]]),
                    f23(3, [[5 * W, 2], [1, W]]),
                    d4(4, [[0, 2], [1, W]]), TT.add)
                e_t.tensor_tensor(At[b][:, 9:10, :], f23(13, [[0, 1], [1, W]]),
                                  d4(4, [[0, 1], [1, W]]), TT.add)
                # arm_r: j = r. Ar[T] = F23[q,t] + D4[r]
                e_r.tensor_tensor(Ar[b][:, 0:3, :], f23(2, [[W, 3], [1, W]]),
                                  d4(1, [[0, 3], [1, W]]), TT.add)
                e_r.tensor_tensor(
                    _ap(Ar[b], 3 * W, [[3 * W, 2], [W, 2], [1, W]]),
                    f23(3, [[5 * W, 2], [W, 2], [1, W]]),
                    d4(2, [[0, 2], [0, 2], [1, W]]), TT.add)
                e_r.tensor_tensor(
                    _ap(Ar[b], 5 * W, [[3 * W, 2], [1, W]]),
                    f23(4, [[5 * W, 2], [1, W]]),
                    d4(3, [[0, 2], [1, W]]), TT.add)
                e_r.tensor_tensor(Ar[b][:, 9:10, :], f23(14, [[0, 1], [1, W]]),
                                  d4(3, [[0, 1], [1, W]]), TT.add)
                # arm_q: j = q. Aq[T] = F23[r,t] + D4[q]
                e_q.tensor_tensor(Aq[b][:, 0:3, :], f23(7, [[W, 3], [1, W]]),
                                  d4(0, [[0, 3], [1, W]]), TT.add)
                e_q.tensor_tensor(
                    _ap(Aq[b], 3 * W, [[3 * W, 2], [W, 2], [1, W]]),
                    f23(13, [[0, 2], [W, 2], [1, W]]),
                    d4(0, [[FS, 2], [0, 2], [1, W]]), TT.add)
                e_q.tensor_tensor(
                    _ap(Aq[b], 5 * W, [[3 * W, 2], [1, W]]),
                    f23(19, [[0, 2], [1, W]]),
                    d4(0, [[FS, 2], [1, W]]), TT.add)
                e_q.tensor_tensor(Aq[b][:, 9:10, :], f23(19, [[0, 1], [1, W]]),
                                  d4(2, [[0, 1], [1, W]]), TT.add)

                g3e.tensor_tensor(G3[b][:], Ar[b][:], Aq[b][:], TT.min)
                g3e.tensor_tensor(G3[b][:], G3[b][:], At[b][:], TT.min)

                # combine: ans[k] = F01[pair k] + g3[9-k]; F01 = Gt[:,0]
                dve.tensor_tensor(
                    ANS[b][:, 0:4, :], _ap(Gt, 1 * W, [[W, 4], [1, W]]),
                    _ap(G3[b], 9 * W, [[-W, 4], [1, W]]), TT.add)
                dve.tensor_tensor(
                    ANS[b][:, 4:7, :], _ap(Gt, 7 * W, [[W, 3], [1, W]]),
                    _ap(G3[b], 5 * W, [[-W, 3], [1, W]]), TT.add)
                dve.tensor_tensor(
                    ANS[b][:, 7:9, :], _ap(Gt, 13 * W, [[W, 2], [1, W]]),
                    _ap(G3[b], 2 * W, [[-W, 2], [1, W]]), TT.add)
                dve.tensor_tensor(
                    ANS[b][:, 9:10, :], _ap(Gt, 19 * W, [[0, 1], [1, W]]),
                    G3[b][:, 0:1, :], TT.add)
                # min tree over the 10 slots
                dve.tensor_tensor(T1[b][:], ANS[b][:, 0:5, :], ANS[b][:, 5:10, :], TT.min)
                dve.tensor_tensor(T2[b][:], T1[b][:, 0:2, :], T1[b][:, 2:4, :], TT.min)
                dve.tensor_tensor(T3[b][:], T2[b][:, 0:1, :], T2[b][:, 1:2, :], TT.min)
                dve.tensor_tensor(RES[:, b], T3[b][:, 0, :], T1[b][:, 4, :], TT.min)

            dve.tensor_reduce(
                PART[:], _ap(RES, 0, [[1, asn_nch * W]]),
                mybir.AxisListType.X, TT.add,
            )
            # collapse to one partition so the output DMA is a single-queue
            # 4-byte transfer (a [128,1] source fans out over 16 queues whose
            # completion crawl costs ~7us at the tail)
            gp.partition_all_reduce(
                PARTR[:], PART[:], 128, bass_isa.ReduceOp.add
            )
            nc.sync.dma_start(out=out_d[:], in_=PARTR[0:1, :])

    nc.compile()
    return nc


_CACHED_RUNNER = None


def _pack_inputs(ta, pa, td, pd, nch):
    """(N, M) f32 x4 -> (NCORES*P, nch, 2, 2, M, CS) f16, chunk-major."""
    CS = FS // nch
    out = np.empty((NCORES * P, nch, 2, 2, M, CS), np.float16)
    for k, (a, b) in enumerate(((ta, pa), (td, pd))):
        a5 = np.asarray(a, np.float32).reshape(NCORES * P, nch, CS, M)
        b5 = np.asarray(b, np.float32).reshape(NCORES * P, nch, CS, M)
        out[:, :, k, 0] = a5.transpose(0, 1, 3, 2)
        out[:, :, k, 1] = b5.transpose(0, 1, 3, 2)
    return out


def _make_runner():
    import jax
    from jax.sharding import Mesh, NamedSharding, PartitionSpec
    from jax.experimental.shard_map import shard_map
    from concourse.bass2jax import (
        _bass_exec_p, install_neuronx_cc_hook, partition_id_tensor,
    )

    nc = build_bass()
    install_neuronx_cc_hook()
    partition_name = nc.partition_id_tensor.name if nc.partition_id_tensor else None
    in_names, out_names, out_avals, zero_outs = [], [], [], []
    for alloc in nc.m.functions[0].allocations:
        if not isinstance(alloc, mybir.MemoryLocationSet):
            continue
        name = alloc.memorylocations[0].name
        if alloc.kind == "ExternalInput":
            if name != partition_name:
                in_names.append(name)
        elif alloc.kind == "ExternalOutput":
            shape = tuple(alloc.tensor_shape)
            dtype = mybir.dt.np(alloc.dtype)
            out_names.append(name)
            out_avals.append(jax.core.ShapedArray(shape, dtype))
            zero_outs.append(np.zeros(shape, dtype))
    n_params = len(in_names)
    all_in_names = in_names + out_names
    if partition_name is not None:
        all_in_names = all_in_names + [partition_name]

    def _body(*args):
        operands = list(args)
        if partition_name is not None:
            operands.append(partition_id_tensor())
        return tuple(_bass_exec_p.bind(
            *operands,
            out_avals=tuple(out_avals),
            in_names=tuple(all_in_names),
            out_names=tuple(out_names),
            lowering_input_output_aliases=(),
            sim_require_finite=True,
            sim_require_nnan=True,
            nc=nc,
        ))

    devices = jax.devices()[:NCORES]
    mesh = Mesh(np.asarray(devices), ("core",))
    in_specs = (PartitionSpec("core"),) * (n_params + len(out_names))
    out_specs = (PartitionSpec("core"),) * len(out_names)
    fn = jax.jit(
        shard_map(_body, mesh=mesh, in_specs=in_specs, out_specs=out_specs,
                  check_rep=False),
        keep_unused=True,
    )
    sharding = NamedSharding(mesh, PartitionSpec("core"))
    concat_zeros = [
        np.zeros((NCORES * z.shape[0], *z.shape[1:]), z.dtype) for z in zero_outs
    ]
    zeros_dev = [jax.device_put(z, sharding) for z in concat_zeros]

    def run(inputs_by_name):
        import jax as _jax
        args = [
            _jax.device_put(np.ascontiguousarray(inputs_by_name[nm]), sharding)
            for nm in in_names
        ]
        outs = fn(*args, *zeros_dev)
        return {nm: np.asarray(outs[i]) for i, nm in enumerate(out_names)}

    return run


def kernel(predictions_angle, targets_angle, predictions_distance, targets_distance):
    global _CACHED_RUNNER
    if _CACHED_RUNNER is None:
        _CACHED_RUNNER = _make_runner()
    out = _CACHED_RUNNER({
        "inp": _pack_inputs(targets_angle, predictions_angle,
                            targets_distance, predictions_distance, NCH),
    })
    total = out["partials"].astype(np.float64).sum()
    return np.asarray(total / N / M, dtype=np.float32)


# revision 23
# speedup vs baseline: 1.0464x; 1.0058x over previous
"""Trainium2 Bass kernel for CartesianLoss (v3, merged-op chunked pipeline).

Loss = mean_n min_perm mean_i ||polar2cart(target_i) - polar2cart(pred_perm(i))||_2

Pure data parallelism over the batch (N=131072) across 8 cores; each core
handles 16384 samples as (128 partitions, 128 samples). Host packs inputs
chunk-major, source-major, fp16, so every device op is contiguous fp16
(DVE 2x packed mode) with no on-device transposes.

Per-op fixed costs (~150ns DVE issue + sem waits) dominate small ops, so v3
merges aggressively:
- one trig pair per chunk (cos/sin into halves of one tile)
- coords for x/y and t/p in ONE tensor_tensor (broadcast AP over 4 planes)
- both outer-difference planes in ONE op
- both squares in ONE op, both pair outer-sum matrices (G01,G23) in ONE op
- triangle fold as 4 dual-F in-place min ops
- per-sample min tree instead of 1x tensor_reduce

Assignment (min over 120 perms) uses meet-in-the-middle: F01/F23 pair mins
via dense 5x5 outer-sum + triangle fold, g3 triples via 3 arms, combine with
reversed-rank access.
"""

import contextlib

import numpy as np

import concourse.bass as bass
import concourse.bass_isa as bass_isa
import concourse.bacc as bacc
import concourse.tile as tile
from concourse import mybir

N = 131072
M = 5
NCORES = 8
NPC = N // NCORES          # samples per core
P = 128                    # partitions
FS = NPC // P              # samples per partition (128)
HALF_PI = 1.5707963267948966

F32 = mybir.dt.float32
F16 = mybir.dt.float16
TT = mybir.AluOpType
AFT = mybir.ActivationFunctionType

# --- tunables -------------------------------------------------------------
NCH = 2                    # front-end sample chunks (divides FS)
ASN_NCH = 1                # assignment blocks (divides FS)
SQ_ENGINE = "split"        # 'act' | 'dve' | 'split' (squares of dx/dy)
ADD_ENGINE = "dve"         # 'dve' | 'gp'    (d2 = dx2 + dy2)
ARMS_ENGINE = "dve"        # 'dve' | 'gp'
G3_ENGINE = "dve"          # 'dve' | 'gp'    (3-way arm min)

TRACE = False


def _ap(t, offset_elems, dims):
    """Manual free-dim AP on tile t: dims = [[step,count],...] (elements)."""
    full = t[:]
    return bass.AP(
        tensor=full.tensor,
        offset=full.offset + offset_elems,
        ap=[full.ap[0]] + [list(d) for d in dims],
    )


def build_bass(loop_iters=None, nch=None, asn_nch=None, sq_engine=None,
               add_engine=None, arms_engine=None, g3_engine=None):
    nch = NCH if nch is None else nch
    asn_nch = ASN_NCH if asn_nch is None else asn_nch
    sq_engine = SQ_ENGINE if sq_engine is None else sq_engine
    add_engine = ADD_ENGINE if add_engine is None else add_engine
    arms_engine = ARMS_ENGINE if arms_engine is None else arms_engine
    g3_engine = G3_ENGINE if g3_engine is None else g3_engine
    CS = FS // nch
    W = FS // asn_nch
    assert FS % nch == 0 and FS % asn_nch == 0

    nc = bacc.Bacc(
        "TRN2", target_bir_lowering=False, debug=False, num_devices=NCORES
    )
    hpi_t = nc.alloc_sbuf_tensor("const-float32-hpi", [P, 1], F32)
    nc.gpsimd.memset(hpi_t.ap(), HALF_PI)
    nc.const_aps.aps[(F32, HALF_PI)] = hpi_t.ap()
    nc.all_engine_barrier()

    in_d = nc.dram_tensor("inp", [P, nch, 2, 2, M, CS], F16, kind="ExternalInput")
    out_d = nc.dram_tensor("partials", [1, 1], F32, kind="ExternalOutput")

    gp = nc.gpsimd
    dve = nc.vector
    eng = {"dve": dve, "gp": gp}
    MCS = M * CS

    with tile.TileContext(nc) as tc:
        with contextlib.ExitStack() as stack:
            if loop_iters is not None:
                stack.enter_context(tc.For_i(0, loop_iters, 1))
            pool = stack.enter_context(tc.tile_pool(name="main", bufs=1))

            def tl(shape, dt, tag):
                return pool.tile(shape, dt, name="t", tag=tag)

            IN = [tl([P, 2, 2, M, CS], F16, f"in{c}") for c in range(nch)]
            ang = [t[:, 0] for t in IN]
            dst = [t[:, 1] for t in IN]
            # TRIG[h]: h=0 cos, h=1 sin, each [2(t/p), M, CS]
            TRIG = [tl([P, 2, 2, M, CS], F16, f"trig{c}") for c in range(nch)]
            CRD = [tl([P, 2, 2, M, CS], F16, f"crd{c}") for c in range(nch)]
            DXY = [tl([P, 2, M, M, CS], F16, f"dxy{c}") for c in range(nch)]
            SQ = [tl([P, 2, M, M, CS], F16, f"sq{c}") for c in range(nch)]
            D2 = [tl([P, M * M, CS], F16, f"d2{c}") for c in range(nch)]
            D = tl([P, M * M, FS], F16, "dfull")
            GT = [tl([P, 2, M, M, W], F16, f"gt{b}") for b in range(asn_nch)]
            At = [tl([P, 10, W], F16, f"at{b}") for b in range(asn_nch)]
            Ar = [tl([P, 10, W], F16, f"ar{b}") for b in range(asn_nch)]
            Aq = [tl([P, 10, W], F16, f"aq{b}") for b in range(asn_nch)]
            G3 = [tl([P, 10, W], F16, f"g3_{b}") for b in range(asn_nch)]
            ANS = [tl([P, 10, W], F16, f"ans{b}") for b in range(asn_nch)]
            T1 = [tl([P, M, W], F16, f"t1_{b}") for b in range(asn_nch)]
            T2 = [tl([P, 2, W], F16, f"t2_{b}") for b in range(asn_nch)]
            T3 = [tl([P, 1, W], F16, f"t3_{b}") for b in range(asn_nch)]
            RES = tl([P, asn_nch, W], F32, "res")
            PART = tl([P, 1], F32, "part")
            PARTR = tl([P, 1], F32, "partr")

            # ---- DMA: one transfer per chunk (ang+dst packed) ----
            for c in range(nch):
                nc.sync.dma_start(out=IN[c][:], in_=in_d[:, c])

            # ---- ACT trig: all chunks first (one table set) ----
            for c in range(nch):
                nc.scalar.activation(TRIG[c][:, 0], ang[c], AFT.Sin, bias=HALF_PI)
                nc.scalar.activation(TRIG[c][:, 1], ang[c], AFT.Sin)


            # ---- DVE front-end per chunk ----
            for c in range(nch):
                # CRD[h,tp,m,s] = TRIG[h,tp,m,s] * dst[tp,m,s]. Chunk 0 is
                # split per h so DVE starts right after the first ACT op
                # (cos0); later chunks' trig is long done, so one merged op.
                if c == 0:
                    for h in range(2):
                        dve.tensor_tensor(
                            _ap(CRD[c], h * 2 * MCS, [[MCS, 2], [1, MCS]]),
                            _ap(TRIG[c], h * 2 * MCS, [[MCS, 2], [1, MCS]]),
                            _ap(IN[c], 2 * MCS, [[MCS, 2], [1, MCS]]),
                            TT.mult,
                        )
                        dve.tensor_tensor(
                            DXY[c][:, h],
                            _ap(CRD[c], h * 2 * MCS, [[CS, M], [0, M], [1, CS]]),
                            _ap(CRD[c], h * 2 * MCS + MCS,
                                [[0, M], [CS, M], [1, CS]]),
                            TT.subtract,
                        )
                else:
                    dve.tensor_tensor(
                        _ap(CRD[c], 0, [[2 * MCS, 2], [MCS, 2], [1, MCS]]),
                        _ap(TRIG[c], 0, [[2 * MCS, 2], [MCS, 2], [1, MCS]]),
                        _ap(IN[c], 2 * MCS, [[0, 2], [MCS, 2], [1, MCS]]),
                        TT.mult,
                    )
                    for h in range(2):
                        dve.tensor_tensor(
                            DXY[c][:, h],
                            _ap(CRD[c], h * 2 * MCS, [[CS, M], [0, M], [1, CS]]),
                            _ap(CRD[c], h * 2 * MCS + MCS,
                                [[0, M], [CS, M], [1, CS]]),
                            TT.subtract,
                        )
                if sq_engine == "dve":
                    dve.tensor_tensor(SQ[c][:], DXY[c][:], DXY[c][:], TT.mult)

            # ---- squares: y-half on ACT (emitted first so ACT streams on),
            #      x-half on DVE when 'split'; d2 add on DVE; sqrt split into
            #      rows 0-3 (gates pair stage) and row 4 (gates arms only) ----
            for c in range(nch):
                if sq_engine == "act":
                    nc.scalar.activation(SQ[c][:], DXY[c][:], AFT.Square)
                elif sq_engine == "split":
                    nc.scalar.activation(SQ[c][:, 1], DXY[c][:, 1], AFT.Square)
            if sq_engine == "split":
                for c in range(nch):
                    dve.tensor_tensor(
                        SQ[c][:, 0], DXY[c][:, 0], DXY[c][:, 0], TT.mult
                    )
            for c in range(nch):
                eng[add_engine].tensor_tensor(
                    D2[c][:], SQ[c][:, 0], SQ[c][:, 1], TT.add
                )
            for r0, nrows in ((0, 10), (10, 10), (20, 5)):
                for c in range(nch):
                    nc.scalar.activation(
                        _ap(D, r0 * FS + c * CS, [[FS, nrows], [1, CS]]),
                        D2[c][:, r0:r0 + nrows], AFT.Sqrt,
                    )

            # ---- assignment per block b over D[:, :, b*W:(b+1)*W] ----
            if arms_engine == "split":
                # arm_t on GpSimd, concurrent with DVE's arm_r/arm_q; g3
                # consumes At last (min(min(Ar,Aq), At)) so the slower Gp
                # path hides behind the DVE ops
                arm_engines = (gp, dve, dve)
            else:
                arm_engines = (eng[arms_engine],) * 3
            g3e = eng[g3_engine]

            for b in range(asn_nch):
                off = b * W
                Gt = GT[b]
                # outer-sums: Gt[f,a,b,s] = D[r0(f),a,s] + D[r1(f),b,s]
                # rows (0,1) for f=0 -> F01, rows (2,3) for f=1 -> F23.
                # Split by sample-half aligned to the sqrt chunks so each
                # half starts as soon as its chunk's rows are sqrted.
                HW_ = W // 2
                for f, (r0, r1) in enumerate(((0, 1), (2, 3))):
                    for ho in (0, HW_):
                        dve.tensor_tensor(
                            _ap(Gt, f * 25 * W + ho,
                                [[M * W, M], [W, M], [1, HW_]]),
                            _ap(D, off + r0 * 5 * FS + ho,
                                [[FS, M], [0, M], [1, HW_]]),
                            _ap(D, off + r1 * 5 * FS + ho,
                                [[0, M], [FS, M], [1, HW_]]),
                            TT.add,
                        )
                # dual triangle fold: F[a<b] = min(G[a,b], G[b,a]), in place
                for a in range(4):
                    n = 4 - a
                    dve.tensor_tensor(
                        _ap(Gt, (6 * a + 1) * W, [[25 * W, 2], [W, n], [1, W]]),
                        _ap(Gt, (6 * a + 1) * W, [[25 * W, 2], [W, n], [1, W]]),
                        _ap(Gt, ((a + 1) * M + a) * W, [[25 * W, 2], [M * W, n], [1, W]]),
                        TT.min,
                    )

                # F23 = Gt[:,1] (offset 25W, slot stride W); D4 = D row 4
                F23o = 25 * W

                def f23(s0, dims):
                    return _ap(Gt, F23o + s0 * W, dims)

                def d4(j0, dims):
                    return _ap(D, off + (20 + j0) * FS, dims)

                e_t, e_r, e_q = arm_engines
                # arm_t: j = t. At[T] = F23[q,r] + D4[t]
                e_t.tensor_tensor(At[b][:, 0:3, :], f23(1, [[0, 3], [1, W]]),
                                  d4(2, [[FS, 3], [1, W]]), TT.add)
                e_t.tensor_tensor(
                    _ap(At[b], 3 * W, [[3 * W, 2], [W, 2], [1, W]]),
                    f23(2, [[5 * W, 2], [0, 2], [1, W]]),
                    d4(3, [[0, 2], [FS, 2], [1, W]]), TT.add)
                e_t.tensor_tensor(
                    _ap(At[b], 5 * W, [[3 * W, 2], [1, W]]),
                    f23(3, [[5 * W, 2], [1, W]]),
                    d4(4, [[0, 2], [1, W]]), TT.add)
                e_t.tensor_tensor(At[b][:, 9:10, :], f23(13, [[0, 1], [1, W]]),
                                  d4(4, [[0, 1], [1, W]]), TT.add)
                # arm_r: j = r. Ar[T] = F23[q,t] + D4[r]
                e_r.tensor_tensor(Ar[b][:, 0:3, :], f23(2, [[W, 3], [1, W]]),
                                  d4(1, [[0, 3], [1, W]]), TT.add)
                e_r.tensor_tensor(
                    _ap(Ar[b], 3 * W, [[3 * W, 2], [W, 2], [1, W]]),
                    f23(3, [[5 * W, 2], [W, 2], [1, W]]),
                    d4(2, [[0, 2], [0, 2], [1, W]]), TT.add)
                e_r.tensor_tensor(
                    _ap(Ar[b], 5 * W, [[3 * W, 2], [1, W]]),
                    f23(4, [[5 * W, 2], [1, W]]),
                    d4(3, [[0, 2], [1, W]]), TT.add)
                e_r.tensor_tensor(Ar[b][:, 9:10, :], f23(14, [[0, 1], [1, W]]),
                                  d4(3, [[0, 1], [1, W]]), TT.add)
                # arm_q: j = q. Aq[T] = F23[r,t] + D4[q]
                e_q.tensor_tensor(Aq[b][:, 0:3, :], f23(7, [[W, 3], [1, W]]),
                                  d4(0, [[0, 3], [1, W]]), TT.add)
                e_q.tensor_tensor(
                    _ap(Aq[b], 3 * W, [[3 * W, 2], [W, 2], [1, W]]),
                    f23(13, [[0, 2], [W, 2], [1, W]]),
                    d4(0, [[FS, 2], [0, 2], [1, W]]), TT.add)
                e_q.tensor_tensor(
                    _ap(Aq[b], 5 * W, [[3 * W, 2], [1, W]]),
                    f23(19, [[0, 2], [1, W]]),
                    d4(0, [[FS, 2], [1, W]]), TT.add)
                e_q.tensor_tensor(Aq[b][:, 9:10, :], f23(19, [[0, 1], [1, W]]),
                                  d4(2, [[0, 1], [1, W]]), TT.add)

                g3e.tensor_tensor(G3[b][:], Ar[b][:], Aq[b][:], TT.min)
                g3e.tensor_tensor(G3[b][:], G3[b][:], At[b][:], TT.min)

                # combine: ans[k] = F01[pair k] + g3[9-k]; F01 = Gt[:,0]
                dve.tensor_tensor(
                    ANS[b][:, 0:4, :], _ap(Gt, 1 * W, [[W, 4], [1, W]]),
                    _ap(G3[b], 9 * W, [[-W, 4], [1, W]]), TT.add)
                dve.tensor_tensor(
                    ANS[b][:, 4:7, :], _ap(Gt, 7 * W, [[W, 3], [1, W]]),
                    _ap(G3[b], 5 * W, [[-W, 3], [1, W]]), TT.add)
                dve.tensor_tensor(
                    ANS[b][:, 7:9, :], _ap(Gt, 13 * W, [[W, 2], [1, W]]),
                    _ap(G3[b], 2 * W, [[-W, 2], [1, W]]), TT.add)
                dve.tensor_tensor(
                    ANS[b][:, 9:10, :], _ap(Gt, 19 * W, [[0, 1], [1, W]]),
                    G3[b][:, 0:1, :], TT.add)
                # min tree over the 10 slots
                dve.tensor_tensor(T1[b][:], ANS[b][:, 0:5, :], ANS[b][:, 5:10, :], TT.min)
                dve.tensor_tensor(T2[b][:], T1[b][:, 0:2, :], T1[b][:, 2:4, :], TT.min)
                dve.tensor_tensor(T3[b][:], T2[b][:, 0:1, :], T2[b][:, 1:2, :], TT.min)
                dve.tensor_tensor(RES[:, b], T3[b][:, 0, :], T1[b][:, 4, :], TT.min)

            dve.tensor_reduce(
                PART[:], _ap(RES, 0, [[1, asn_nch * W]]),
                mybir.AxisListType.X, TT.add,
            )
            # collapse to one partition so the output DMA is a single-queue
            # 4-byte transfer (a [128,1] source fans out over 16 queues whose
            # completion crawl costs ~7us at the tail)
            gp.partition_all_reduce(
                PARTR[:], PART[:], 128, bass_isa.ReduceOp.add
            )
            nc.sync.dma_start(out=out_d[:], in_=PARTR[0:1, :])

    nc.compile()
    return nc


_CACHED_RUNNER = None


def _pack_inputs(ta, pa, td, pd, nch):
    """(N, M) f32 x4 -> (NCORES*P, nch, 2, 2, M, CS) f16, chunk-major."""
    CS = FS // nch
    out = np.empty((NCORES * P, nch, 2, 2, M, CS), np.float16)
    for k, (a, b) in enumerate(((ta, pa), (td, pd))):
        a5 = np.asarray(a, np.float32).reshape(NCORES * P, nch, CS, M)
        b5 = np.asarray(b, np.float32).reshape(NCORES * P, nch, CS, M)
        out[:, :, k, 0] = a5.transpose(0, 1, 3, 2)
        out[:, :, k, 1] = b5.transpose(0, 1, 3, 2)
    return out


def _make_runner():
    import jax
    from jax.sharding import Mesh, NamedSharding, PartitionSpec
    from jax.experimental.shard_map import shard_map
    from concourse.bass2jax import (
        _bass_exec_p, install_neuronx_cc_hook, partition_id_tensor,
    )

    nc = build_bass()
    install_neuronx_cc_hook()
    partition_name = nc.partition_id_tensor.name if nc.partition_id_tensor else None
    in_names, out_names, out_avals, zero_outs = [], [], [], []
    for alloc in nc.m.functions[0].allocations:
        if not isinstance(alloc, mybir.MemoryLocationSet):
            continue
        name = alloc.memorylocations[0].name
        if alloc.kind == "ExternalInput":
            if name != partition_name:
                in_names.append(name)
        elif alloc.kind == "ExternalOutput":
            shape = tuple(alloc.tensor_shape)
            dtype = mybir.dt.np(alloc.dtype)
            out_names.append(name)
            out_avals.append(jax.core.ShapedArray(shape, dtype))
            zero_outs.append(np.zeros(shape, dtype))
    n_params = len(in_names)
    all_in_names = in_names + out_names
    if partition_name is not None:
        all_in_names = all_in_names + [partition_name]

    def _body(*args):
        operands = list(args)
        if partition_name is not None:
            operands.append(partition_id_tensor())
        return tuple(_bass_exec_p.bind(
            *operands,
            out_avals=tuple(out_avals),
            in_names=tuple(all_in_names),
            out_names=tuple(out_names),
            lowering_input_output_aliases=(),
            sim_require_finite=True,
            sim_require_nnan=True,
            nc=nc,
        ))

    devices = jax.devices()[:NCORES]
    mesh = Mesh(np.asarray(devices), ("core",))
    in_specs = (PartitionSpec("core"),) * (n_params + len(out_names))
    out_specs = (PartitionSpec("core"),) * len(out_names)
    fn = jax.jit(
        shard_map(_body, mesh=mesh, in_specs=in_specs, out_specs=out_specs,
                  check_rep=False),
        keep_unused=True,
    )
    sharding = NamedSharding(mesh, PartitionSpec("core"))
    concat_zeros = [
        np.zeros((NCORES * z.shape[0], *z.shape[1:]), z.dtype) for z in zero_outs
    ]
    zeros_dev = [jax.device_put(z, sharding) for z in concat_zeros]

    def run(inputs_by_name):
        import jax as _jax
        args = [
            _jax.device_put(np.ascontiguousarray(inputs_by_name[nm]), sharding)
            for nm in in_names
        ]
        outs = fn(*args, *zeros_dev)
        return {nm: np.asarray(outs[i]) for i, nm in enumerate(out_names)}

    return run


def kernel(predictions_angle, targets_angle, predictions_distance, targets_distance):
    global _CACHED_RUNNER
    if _CACHED_RUNNER is None:
        _CACHED_RUNNER = _make_runner()
    out = _CACHED_RUNNER({
        "inp": _pack_inputs(targets_angle, predictions_angle,
                            targets_distance, predictions_distance, NCH),
    })
    total = out["partials"].astype(np.float64).sum()
    return np.asarray(total / N / M, dtype=np.float32)


# revision 24
# speedup vs baseline: 1.0758x; 1.0281x over previous
"""Trainium2 Bass kernel for CartesianLoss (v3, merged-op chunked pipeline).

Loss = mean_n min_perm mean_i ||polar2cart(target_i) - polar2cart(pred_perm(i))||_2

Pure data parallelism over the batch (N=131072) across 8 cores; each core
handles 16384 samples as (128 partitions, 128 samples). Host packs inputs
chunk-major, source-major, fp16, so every device op is contiguous fp16
(DVE 2x packed mode) with no on-device transposes.

Per-op fixed costs (~150ns DVE issue + sem waits) dominate small ops, so v3
merges aggressively:
- one trig pair per chunk (cos/sin into halves of one tile)
- coords for x/y and t/p in ONE tensor_tensor (broadcast AP over 4 planes)
- both outer-difference planes in ONE op
- both squares in ONE op, both pair outer-sum matrices (G01,G23) in ONE op
- triangle fold as 4 dual-F in-place min ops
- per-sample min tree instead of 1x tensor_reduce

Assignment (min over 120 perms) uses meet-in-the-middle: F01/F23 pair mins
via dense 5x5 outer-sum + triangle fold, g3 triples via 3 arms, combine with
reversed-rank access.
"""

import contextlib

import numpy as np

import concourse.bass as bass
import concourse.bass_isa as bass_isa
import concourse.bacc as bacc
import concourse.tile as tile
from concourse import mybir

N = 131072
M = 5
NCORES = 8
NPC = N // NCORES          # samples per core
P = 128                    # partitions
FS = NPC // P              # samples per partition (128)
HALF_PI = 1.5707963267948966

F32 = mybir.dt.float32
F16 = mybir.dt.float16
TT = mybir.AluOpType
AFT = mybir.ActivationFunctionType

# --- tunables -------------------------------------------------------------
NCH = 2                    # front-end sample chunks (divides FS)
ASN_NCH = 1                # assignment blocks (divides FS)
SQ_ENGINE = "split"        # 'act' | 'dve' | 'split' (squares of dx/dy)
ADD_ENGINE = "dve"         # 'dve' | 'gp'    (d2 = dx2 + dy2)
ARMS_ENGINE = "dve"        # 'dve' | 'gp'
G3_ENGINE = "dve"          # 'dve' | 'gp'    (3-way arm min)

TRACE = False


def _ap(t, offset_elems, dims):
    """Manual free-dim AP on tile t: dims = [[step,count],...] (elements)."""
    full = t[:]
    return bass.AP(
        tensor=full.tensor,
        offset=full.offset + offset_elems,
        ap=[full.ap[0]] + [list(d) for d in dims],
    )


def build_bass(loop_iters=None, nch=None, asn_nch=None, sq_engine=None,
               add_engine=None, arms_engine=None, g3_engine=None):
    nch = NCH if nch is None else nch
    asn_nch = ASN_NCH if asn_nch is None else asn_nch
    sq_engine = SQ_ENGINE if sq_engine is None else sq_engine
    add_engine = ADD_ENGINE if add_engine is None else add_engine
    arms_engine = ARMS_ENGINE if arms_engine is None else arms_engine
    g3_engine = G3_ENGINE if g3_engine is None else g3_engine
    CS = FS // nch
    W = FS // asn_nch
    assert FS % nch == 0 and FS % asn_nch == 0

    nc = bacc.Bacc(
        "TRN2", target_bir_lowering=False, debug=False, num_devices=NCORES
    )
    hpi_t = nc.alloc_sbuf_tensor("const-float32-hpi", [P, 1], F32)
    nc.gpsimd.memset(hpi_t.ap(), HALF_PI)
    nc.const_aps.aps[(F32, HALF_PI)] = hpi_t.ap()
    nc.all_engine_barrier()

    in_d = nc.dram_tensor("inp", [P, nch, 2, 2, M, CS], F16, kind="ExternalInput")
    out_d = nc.dram_tensor("partials", [1, 1], F32, kind="ExternalOutput")

    gp = nc.gpsimd
    dve = nc.vector
    eng = {"dve": dve, "gp": gp}
    MCS = M * CS

    with tile.TileContext(nc) as tc:
        with contextlib.ExitStack() as stack:
            if loop_iters is not None:
                stack.enter_context(tc.For_i(0, loop_iters, 1))
            pool = stack.enter_context(tc.tile_pool(name="main", bufs=1))

            def tl(shape, dt, tag):
                return pool.tile(shape, dt, name="t", tag=tag)

            IN = [tl([P, 2, 2, M, CS], F16, f"in{c}") for c in range(nch)]
            ang = [t[:, 0] for t in IN]
            dst = [t[:, 1] for t in IN]
            # TRIG[h]: h=0 cos, h=1 sin, each [2(t/p), M, CS]
            TRIG = [tl([P, 2, 2, M, CS], F16, f"trig{c}") for c in range(nch)]
            CRD = [tl([P, 2, 2, M, CS], F16, f"crd{c}") for c in range(nch)]
            DXY = [tl([P, 2, M, M, CS], F16, f"dxy{c}") for c in range(nch)]
            SQ = [tl([P, 2, M, M, CS], F16, f"sq{c}") for c in range(nch)]
            D2 = [tl([P, M * M, CS], F16, f"d2{c}") for c in range(nch)]
            D = tl([P, M * M, FS], F16, "dfull")
            GT = [tl([P, 2, M, M, W], F16, f"gt{b}") for b in range(asn_nch)]
            At = [tl([P, 10, W], F16, f"at{b}") for b in range(asn_nch)]
            Ar = [tl([P, 10, W], F16, f"ar{b}") for b in range(asn_nch)]
            Aq = [tl([P, 10, W], F16, f"aq{b}") for b in range(asn_nch)]
            G3 = [tl([P, 10, W], F16, f"g3_{b}") for b in range(asn_nch)]
            ANS = [tl([P, 10, W], F16, f"ans{b}") for b in range(asn_nch)]
            T1 = [tl([P, M, W], F16, f"t1_{b}") for b in range(asn_nch)]
            T2 = [tl([P, 2, W], F16, f"t2_{b}") for b in range(asn_nch)]
            T3 = [tl([P, 1, W], F16, f"t3_{b}") for b in range(asn_nch)]
            RES = tl([P, asn_nch, W], F32, "res")
            PART = tl([P, 1], F32, "part")
            PARTR = tl([P, 1], F32, "partr")

            # ---- DMA: one transfer per chunk (ang+dst packed) ----
            for c in range(nch):
                nc.sync.dma_start(out=IN[c][:], in_=in_d[:, c])

            # ---- ACT trig: all chunks first (one table set) ----
            for c in range(nch):
                nc.scalar.activation(TRIG[c][:, 0], ang[c], AFT.Sin, bias=HALF_PI)
                nc.scalar.activation(TRIG[c][:, 1], ang[c], AFT.Sin)


            # ---- DVE front-end per chunk ----
            for c in range(nch):
                # CRD[h,tp,m,s] = TRIG[h,tp,m,s] * dst[tp,m,s]. Chunk 0 is
                # split per h so DVE starts right after the first ACT op
                # (cos0); later chunks' trig is long done, so one merged op.
                if c == 0:
                    for h in range(2):
                        dve.tensor_tensor(
                            _ap(CRD[c], h * 2 * MCS, [[MCS, 2], [1, MCS]]),
                            _ap(TRIG[c], h * 2 * MCS, [[MCS, 2], [1, MCS]]),
                            _ap(IN[c], 2 * MCS, [[MCS, 2], [1, MCS]]),
                            TT.mult,
                        )
                        dve.tensor_tensor(
                            DXY[c][:, h],
                            _ap(CRD[c], h * 2 * MCS, [[CS, M], [0, M], [1, CS]]),
                            _ap(CRD[c], h * 2 * MCS + MCS,
                                [[0, M], [CS, M], [1, CS]]),
                            TT.subtract,
                        )
                else:
                    dve.tensor_tensor(
                        _ap(CRD[c], 0, [[2 * MCS, 2], [MCS, 2], [1, MCS]]),
                        _ap(TRIG[c], 0, [[2 * MCS, 2], [MCS, 2], [1, MCS]]),
                        _ap(IN[c], 2 * MCS, [[0, 2], [MCS, 2], [1, MCS]]),
                        TT.mult,
                    )
                    for h in (1, 0):
                        dve.tensor_tensor(
                            DXY[c][:, h],
                            _ap(CRD[c], h * 2 * MCS, [[CS, M], [0, M], [1, CS]]),
                            _ap(CRD[c], h * 2 * MCS + MCS,
                                [[0, M], [CS, M], [1, CS]]),
                            TT.subtract,
                        )
                if sq_engine == "dve":
                    dve.tensor_tensor(SQ[c][:], DXY[c][:], DXY[c][:], TT.mult)

            # ---- squares: y-half on ACT (emitted first so ACT streams on),
            #      x-half on DVE when 'split'; d2 add on DVE; sqrt split into
            #      rows 0-3 (gates pair stage) and row 4 (gates arms only) ----
            for c in range(nch):
                if sq_engine == "act":
                    nc.scalar.activation(SQ[c][:], DXY[c][:], AFT.Square)
                elif sq_engine == "split":
                    nc.scalar.activation(SQ[c][:, 1], DXY[c][:, 1], AFT.Square)
            for c in range(nch):
                if sq_engine == "split":
                    dve.tensor_tensor(
                        SQ[c][:, 0], DXY[c][:, 0], DXY[c][:, 0], TT.mult
                    )
                eng[add_engine].tensor_tensor(
                    D2[c][:], SQ[c][:, 0], SQ[c][:, 1], TT.add
                )
            for r0, nrows in ((0, 10), (10, 10), (20, 5)):
                for c in range(nch):
                    nc.scalar.activation(
                        _ap(D, r0 * FS + c * CS, [[FS, nrows], [1, CS]]),
                        D2[c][:, r0:r0 + nrows], AFT.Sqrt,
                    )

            # ---- assignment per block b over D[:, :, b*W:(b+1)*W] ----
            if arms_engine == "split":
                # arm_t on GpSimd, concurrent with DVE's arm_r/arm_q; g3
                # consumes At last (min(min(Ar,Aq), At)) so the slower Gp
                # path hides behind the DVE ops
                arm_engines = (gp, dve, dve)
            else:
                arm_engines = (eng[arms_engine],) * 3
            g3e = eng[g3_engine]

            for b in range(asn_nch):
                off = b * W
                Gt = GT[b]
                # outer-sums: Gt[f,a,b,s] = D[r0(f),a,s] + D[r1(f),b,s]
                # rows (0,1) for f=0 -> F01, rows (2,3) for f=1 -> F23.
                # Split by sample-half aligned to the sqrt chunks so each
                # half starts as soon as its chunk's rows are sqrted.
                HW_ = W // 2
                for f, (r0, r1) in enumerate(((0, 1), (2, 3))):
                    for ho in (0, HW_):
                        dve.tensor_tensor(
                            _ap(Gt, f * 25 * W + ho,
                                [[M * W, M], [W, M], [1, HW_]]),
                            _ap(D, off + r0 * 5 * FS + ho,
                                [[FS, M], [0, M], [1, HW_]]),
                            _ap(D, off + r1 * 5 * FS + ho,
                                [[0, M], [FS, M], [1, HW_]]),
                            TT.add,
                        )
                # dual triangle fold: F[a<b] = min(G[a,b], G[b,a]), in place
                for a in range(4):
                    n = 4 - a
                    dve.tensor_tensor(
                        _ap(Gt, (6 * a + 1) * W, [[25 * W, 2], [W, n], [1, W]]),
                        _ap(Gt, (6 * a + 1) * W, [[25 * W, 2], [W, n], [1, W]]),
                        _ap(Gt, ((a + 1) * M + a) * W, [[25 * W, 2], [M * W, n], [1, W]]),
                        TT.min,
                    )

                # F23 = Gt[:,1] (offset 25W, slot stride W); D4 = D row 4
                F23o = 25 * W

                def f23(s0, dims):
                    return _ap(Gt, F23o + s0 * W, dims)

                def d4(j0, dims):
                    return _ap(D, off + (20 + j0) * FS, dims)

                e_t, e_r, e_q = arm_engines
                # arm_t: j = t. At[T] = F23[q,r] + D4[t]
                e_t.tensor_tensor(At[b][:, 0:3, :], f23(1, [[0, 3], [1, W]]),
                                  d4(2, [[FS, 3], [1, W]]), TT.add)
                e_t.tensor_tensor(
                    _ap(At[b], 3 * W, [[3 * W, 2], [W, 2], [1, W]]),
                    f23(2, [[5 * W, 2], [0, 2], [1, W]]),
                    d4(3, [[0, 2], [FS, 2], [1, W]]), TT.add)
                e_t.tensor_tensor(
                    _ap(At[b], 5 * W, [[3 * W, 2], [1, W]]),
                    f23(3, [[5 * W, 2], [1, W]]),
                    d4(4, [[0, 2], [1, W]]), TT.add)
                e_t.tensor_tensor(At[b][:, 9:10, :], f23(13, [[0, 1], [1, W]]),
                                  d4(4, [[0, 1], [1, W]]), TT.add)
                # arm_r: j = r. Ar[T] = F23[q,t] + D4[r]
                e_r.tensor_tensor(Ar[b][:, 0:3, :], f23(2, [[W, 3], [1, W]]),
                                  d4(1, [[0, 3], [1, W]]), TT.add)
                e_r.tensor_tensor(
                    _ap(Ar[b], 3 * W, [[3 * W, 2], [W, 2], [1, W]]),
                    f23(3, [[5 * W, 2], [W, 2], [1, W]]),
                    d4(2, [[0, 2], [0, 2], [1, W]]), TT.add)
                e_r.tensor_tensor(
                    _ap(Ar[b], 5 * W, [[3 * W, 2], [1, W]]),
                    f23(4, [[5 * W, 2], [1, W]]),
                    d4(3, [[0, 2], [1, W]]), TT.add)
                e_r.tensor_tensor(Ar[b][:, 9:10, :], f23(14, [[0, 1], [1, W]]),
                                  d4(3, [[0, 1], [1, W]]), TT.add)
                # arm_q: j = q. Aq[T] = F23[r,t] + D4[q]
                e_q.tensor_tensor(Aq[b][:, 0:3, :], f23(7, [[W, 3], [1, W]]),
                                  d4(0, [[0, 3], [1, W]]), TT.add)
                e_q.tensor_tensor(
                    _ap(Aq[b], 3 * W, [[3 * W, 2], [W, 2], [1, W]]),
                    f23(13, [[0, 2], [W, 2], [1, W]]),
                    d4(0, [[FS, 2], [0, 2], [1, W]]), TT.add)
                e_q.tensor_tensor(
                    _ap(Aq[b], 5 * W, [[3 * W, 2], [1, W]]),
                    f23(19, [[0, 2], [1, W]]),
                    d4(0, [[FS, 2], [1, W]]), TT.add)
                e_q.tensor_tensor(Aq[b][:, 9:10, :], f23(19, [[0, 1], [1, W]]),
                                  d4(2, [[0, 1], [1, W]]), TT.add)

                g3e.tensor_tensor(G3[b][:], Ar[b][:], Aq[b][:], TT.min)
                g3e.tensor_tensor(G3[b][:], G3[b][:], At[b][:], TT.min)

                # combine: ans[k] = F01[pair k] + g3[9-k]; F01 = Gt[:,0]
                dve.tensor_tensor(
                    ANS[b][:, 0:4, :], _ap(Gt, 1 * W, [[W, 4], [1, W]]),
                    _ap(G3[b], 9 * W, [[-W, 4], [1, W]]), TT.add)
                dve.tensor_tensor(
                    ANS[b][:, 4:7, :], _ap(Gt, 7 * W, [[W, 3], [1, W]]),
                    _ap(G3[b], 5 * W, [[-W, 3], [1, W]]), TT.add)
                dve.tensor_tensor(
                    ANS[b][:, 7:9, :], _ap(Gt, 13 * W, [[W, 2], [1, W]]),
                    _ap(G3[b], 2 * W, [[-W, 2], [1, W]]), TT.add)
                dve.tensor_tensor(
                    ANS[b][:, 9:10, :], _ap(Gt, 19 * W, [[0, 1], [1, W]]),
                    G3[b][:, 0:1, :], TT.add)
                # min tree over the 10 slots
                dve.tensor_tensor(T1[b][:], ANS[b][:, 0:5, :], ANS[b][:, 5:10, :], TT.min)
                dve.tensor_tensor(T2[b][:], T1[b][:, 0:2, :], T1[b][:, 2:4, :], TT.min)
                dve.tensor_tensor(T3[b][:], T2[b][:, 0:1, :], T2[b][:, 1:2, :], TT.min)
                dve.tensor_tensor(RES[:, b], T3[b][:, 0, :], T1[b][:, 4, :], TT.min)

            dve.tensor_reduce(
                PART[:], _ap(RES, 0, [[1, asn_nch * W]]),
                mybir.AxisListType.X, TT.add,
            )
            # collapse to one partition so the output DMA is a single-queue
            # 4-byte transfer (a [128,1] source fans out over 16 queues whose
            # completion crawl costs ~7us at the tail)
            gp.partition_all_reduce(
                PARTR[:], PART[:], 128, bass_isa.ReduceOp.add
            )
            nc.sync.dma_start(out=out_d[:], in_=PARTR[0:1, :])

    nc.compile()
    return nc


_CACHED_RUNNER = None


def _pack_inputs(ta, pa, td, pd, nch):
    """(N, M) f32 x4 -> (NCORES*P, nch, 2, 2, M, CS) f16, chunk-major."""
    CS = FS // nch
    out = np.empty((NCORES * P, nch, 2, 2, M, CS), np.float16)
    for k, (a, b) in enumerate(((ta, pa), (td, pd))):
        a5 = np.asarray(a, np.float32).reshape(NCORES * P, nch, CS, M)
        b5 = np.asarray(b, np.float32).reshape(NCORES * P, nch, CS, M)
        out[:, :, k, 0] = a5.transpose(0, 1, 3, 2)
        out[:, :, k, 1] = b5.transpose(0, 1, 3, 2)
    return out


def _make_runner():
    import jax
    from jax.sharding import Mesh, NamedSharding, PartitionSpec
    from jax.experimental.shard_map import shard_map
    from concourse.bass2jax import (
        _bass_exec_p, install_neuronx_cc_hook, partition_id_tensor,
    )

    nc = build_bass()
    install_neuronx_cc_hook()
    partition_name = nc.partition_id_tensor.name if nc.partition_id_tensor else None
    in_names, out_names, out_avals, zero_outs = [], [], [], []
    for alloc in nc.m.functions[0].allocations:
        if not isinstance(alloc, mybir.MemoryLocationSet):
            continue
        name = alloc.memorylocations[0].name
        if alloc.kind == "ExternalInput":
            if name != partition_name:
                in_names.append(name)
        elif alloc.kind == "ExternalOutput":
            shape = tuple(alloc.tensor_shape)
            dtype = mybir.dt.np(alloc.dtype)
            out_names.append(name)
            out_avals.append(jax.core.ShapedArray(shape, dtype))
            zero_outs.append(np.zeros(shape, dtype))
    n_params = len(in_names)
    all_in_names = in_names + out_names
    if partition_name is not None:
        all_in_names = all_in_names + [partition_name]

    def _body(*args):
        operands = list(args)
        if partition_name is not None:
            operands.append(partition_id_tensor())
        return tuple(_bass_exec_p.bind(
            *operands,
            out_avals=tuple(out_avals),
            in_names=tuple(all_in_names),
            out_names=tuple(out_names),
            lowering_input_output_aliases=(),
            sim_require_finite=True,
            sim_require_nnan=True,
            nc=nc,
        ))

    devices = jax.devices()[:NCORES]
    mesh = Mesh(np.asarray(devices), ("core",))
    in_specs = (PartitionSpec("core"),) * (n_params + len(out_names))
    out_specs = (PartitionSpec("core"),) * len(out_names)
    fn = jax.jit(
        shard_map(_body, mesh=mesh, in_specs=in_specs, out_specs=out_specs,
                  check_rep=False),
        keep_unused=True,
    )
    sharding = NamedSharding(mesh, PartitionSpec("core"))
    concat_zeros = [
        np.zeros((NCORES * z.shape[0], *z.shape[1:]), z.dtype) for z in zero_outs
    ]
    zeros_dev = [jax.device_put(z, sharding) for z in concat_zeros]

    def run(inputs_by_name):
        import jax as _jax
        args = [
            _jax.device_put(np.ascontiguousarray(inputs_by_name[nm]), sharding)
            for nm in in_names
        ]
        outs = fn(*args, *zeros_dev)
        return {nm: np.asarray(outs[i]) for i, nm in enumerate(out_names)}

    return run


def kernel(predictions_angle, targets_angle, predictions_distance, targets_distance):
    global _CACHED_RUNNER
    if _CACHED_RUNNER is None:
        _CACHED_RUNNER = _make_runner()
    out = _CACHED_RUNNER({
        "inp": _pack_inputs(targets_angle, predictions_angle,
                            targets_distance, predictions_distance, NCH),
    })
    total = out["partials"].astype(np.float64).sum()
    return np.asarray(total / N / M, dtype=np.float32)
